# revision 1
# baseline (speedup 1.0000x reference)
"""Multi-head causal attention (B=2, S=2048, DIM=2048, H=16, HD=128) with RoPE,
distributed over 8 Trainium2 NeuronCores.

Sharding: data-parallel over batch (2) x tensor-parallel over head groups (4):
core = b*4 + g handles batch b, heads [4g, 4g+4). Each core computes
Q/K/V projections for its head group (bf16 matmuls, fp32 psum), applies RoPE,
runs causal flash-style attention entirely in "d-major" layouts (no on-device
transposes), applies the output projection rows for its heads, and returns a
partial (S, DIM) output. Host sums the 4 partials per batch (row-parallel wo).

Layout tricks:
  - x is fed pre-transposed (xT, dim-major): serves as lhsT for V and as the
    moving operand for Q^T/K^T, so projections directly produce d-major Q^T/K^T.
  - Inputs land in [128, 16*512] "big tiles" (contraction blocks side by side
    in the free dim) via a few 3D-access-pattern DMAs: DMA issue rate, not
    bandwidth, paced the projection phase with per-block transfers.
  - RoPE in d-major: rot = raw*C + swap(raw)*S_signed; the pair-swap (d ^ 1)
    is two stride-2-partition SBUF->SBUF DMAs on the gpsimd queue.
  - Scores are computed transposed (S^T tiles, j on partitions), softmax is
    max-free (scores ~ N(0,1): exp never overflows). The causal mask is a
    tiny triT^T @ I matmul appended to the scores PSUM accumulation group.
    Softmax denominators accumulate in a [1,512] PSUM via ones-column
    matmuls; normalization = scalar copy off PSUM -> gpsimd
    partition_broadcast -> full-width DVE reciprocal -> DVE multiply
    (a [1,512] DVE reciprocal hits a 1-partition slow path, ~3.4us).
  - The phase-B inner loop is software-pipelined by one j-tile; PSUM banks:
    3 shared projection/O-accumulator slots + 3 score slots + 2 L slots.
  - P^T tiles feed P@V directly; attention output lands d-major (O^T), which
    is exactly the stationary operand the output projection needs.
  - Partial outputs are written bf16 (host sums partials in fp32), batched
    as one [128, 2048] DMA per s-tile on the gpsimd queue.
"""

import numpy as np
import ml_dtypes

import concourse.bacc as bacc
import concourse.mybir as mybir
import concourse.tile as tile
from concourse import bass_isa
from concourse.bass_utils import run_bass_kernel_spmd

B, S, DIM, H, HD = 2, 2048, 2048, 16, 128
NCORES = 8
GROUPS = 4               # head groups (tensor-parallel)
HPC = H // GROUPS        # 4 heads per core
GD = HPC * HD            # 512 dims per group
NKT = DIM // 128         # 16 contraction tiles
NSB = S // 512           # 4 s blocks
NIB = S // 512           # 4 i blocks
F32 = mybir.dt.float32
BF16 = mybir.dt.bfloat16
BF = ml_dtypes.bfloat16
NEG = -1e9

_CACHE = {}


def _build():
    nc = bacc.Bacc("TRN2", target_bir_lowering=False, debug=False,
                   num_devices=NCORES)
    xT = nc.dram_tensor("xT", [DIM, S], BF16, kind="ExternalInput").ap()
    wq = nc.dram_tensor("wq", [DIM, GD], BF16, kind="ExternalInput").ap()
    wk = nc.dram_tensor("wk", [DIM, GD], BF16, kind="ExternalInput").ap()
    wv = nc.dram_tensor("wv", [DIM, GD], BF16, kind="ExternalInput").ap()
    wo = nc.dram_tensor("wo", [GD, DIM], BF16, kind="ExternalInput").ap()
    ropeC = nc.dram_tensor("ropeC", [HD, S], BF16, kind="ExternalInput").ap()
    ropeS = nc.dram_tensor("ropeS", [HD, S], BF16, kind="ExternalInput").ap()
    triT = nc.dram_tensor("triT", [128, 128], BF16, kind="ExternalInput").ap()
    ident = nc.dram_tensor("ident", [128, 128], BF16, kind="ExternalInput").ap()
    pmat = nc.dram_tensor("pmat", [128, 128], BF16, kind="ExternalInput").ap()
    out = nc.dram_tensor("out", [S, DIM], BF16, kind="ExternalOutput").ap()

    from contextlib import ExitStack
    with tile.TileContext(nc) as tc:
        with ExitStack() as ctx:
            pool = lambda *a, **k: ctx.enter_context(tc.tile_pool(*a, **k))
            wpool = pool(name="wpool", bufs=1)
            xpool = pool(name="xpool", bufs=2)
            qkpool = pool(name="qkpool", bufs=HPC)
            vpool = pool(name="vpool", bufs=S // 128)
            otpool = pool(name="otpool", bufs=HPC * NIB)
            wopool = pool(name="wopool", bufs=HPC * 4)
            cpool = pool(name="cpool", bufs=1)
            stage = pool(name="stage", bufs=3)
            tpool = pool(name="tpool", bufs=2)
            ptpool = pool(name="ptpool", bufs=6)
            lrpool = pool(name="lrpool", bufs=3)
            bcpool = pool(name="bcpool", bufs=2)
            copool = pool(name="copool", bufs=2)
            ps_mm = pool(name="ps_mm", bufs=3, space="PSUM")
            ps_l = pool(name="ps_l", bufs=2, space="PSUM")
            ps_st = pool(name="ps_st", bufs=3, space="PSUM")
            # ---- weights / constants: emission order = DMA priority.
            # First x-strip (sb=0) + wq interleaved so Q-proj starts asap;
            # then wk, wv; consts; wo last (phase C only).
            # big-tile layout: all 16 contraction blocks side by side in the
            # free dim ([p, kt*512 + m] = w[kt*128 + p, m]); one or two DMAs
            # per matrix instead of 16 (DMA issue rate, not BW, paced phase A)
            def load_w3d(dst, srcap, eng0, eng1, halves=2):
                nk = NKT // halves
                for hf in range(halves):
                    eng = eng0 if hf % 2 == 0 else eng1
                    eng.dma_start(
                        dst[:, hf * nk * 512:(hf + 1) * nk * 512].rearrange(
                            "p (k m) -> p k m", k=nk),
                        srcap[hf * nk * 128:(hf + 1) * nk * 128, :].rearrange(
                            "(k p) m -> p k m", p=128),
                    )

            wq_all = wpool.tile([128, NKT * 512], BF16, tag="wq")
            xt0 = xpool.tile([128, NKT * 512], BF16, tag="xtb", name="xtb0")
            load_w3d(xt0, xT[:, 0:512], nc.gpsimd, nc.gpsimd, halves=4)
            load_w3d(wq_all, wq, nc.sync, nc.sync, halves=4)
            ropeC_t = cpool.tile([HD, S], BF16, tag="ropeC")
            nc.sync.dma_start(ropeC_t[:], ropeC[:, :])
            ropeS_t = cpool.tile([HD, S], BF16, tag="ropeS")
            nc.sync.dma_start(ropeS_t[:], ropeS[:, :])
            triT_t = cpool.tile([128, 128], BF16, tag="triT")
            nc.sync.dma_start(triT_t[:], triT[:, :])
            ident_t = cpool.tile([128, 128], BF16, tag="ident")
            nc.sync.dma_start(ident_t[:], ident[:, :])
            ones_col = cpool.tile([128, 1], BF16, tag="ones_col")
            nc.vector.memset(ones_col[:], 1.0)
            wk_all = wpool.tile([128, NKT * 512], BF16, tag="wk")
            load_w3d(wk_all, wk, nc.sync, nc.sync)
            wv_all = wpool.tile([128, NKT * 512], BF16, tag="wv")
            load_w3d(wv_all, wv, nc.sync, nc.sync)

            # persistent activations (bf16)
            qt_t = [qkpool.tile([128, S], BF16, tag="qt", name=f"qt{h}") for h in range(HPC)]
            kt_t = [qkpool.tile([128, S], BF16, tag="kt", name=f"ktt{h}") for h in range(HPC)]
            v_t = [vpool.tile([128, GD], BF16, tag="v", name=f"v{st}") for st in range(S // 128)]
            ot_t = {}
            for h in range(HPC):
                for ib in range(NIB):
                    ot_t[(h, ib)] = otpool.tile([128, 512], BF16, tag="ot", name=f"ot{h}_{ib}")

            # ---- phase A: projections + rope ----
            for sb in range(NSB):
                s0 = sb * 512
                if sb == 0:
                    xt = xt0
                else:
                    xt = xpool.tile([128, NKT * 512], BF16, tag="xtb",
                                    name=f"xtb{sb}")
                    load_w3d(xt, xT[:, s0:s0 + 512], nc.sync, nc.sync,
                             halves=1)

                for w_all, dst in ((wq_all, qt_t), (wk_all, kt_t)):
                    for h in range(HPC):
                        pmm = ps_mm.tile([128, 512], F32, tag="mm")
                        for kt in range(NKT):
                            k0 = kt * 512
                            nc.tensor.matmul(
                                pmm[:],
                                w_all[:, k0 + h * 128:k0 + (h + 1) * 128],
                                xt[:, k0:k0 + 512],
                                start=(kt == 0), stop=(kt == NKT - 1),
                            )
                        raw = stage.tile([128, 512], BF16, tag="raw")
                        nc.scalar.copy(raw[:], pmm[:])
                        # pair-swap (d ^ 1) via two stride-2-partition
                        # SBUF->SBUF DMAs on the gpsimd queue
                        sw = stage.tile([128, 512], BF16, tag="sw")
                        nc.gpsimd.dma_start(sw[0:128:2, :], raw[1:128:2, :])
                        nc.gpsimd.dma_start(sw[1:128:2, :], raw[0:128:2, :])
                        t1 = tpool.tile([128, 512], BF16, tag="t1")
                        nc.vector.tensor_mul(t1[:], raw[:],
                                             ropeC_t[:, s0:s0 + 512])
                        t2 = tpool.tile([128, 512], BF16, tag="t2")
                        nc.vector.tensor_mul(t2[:], sw[:],
                                             ropeS_t[:, s0:s0 + 512])
                        nc.vector.tensor_add(dst[h][:, s0:s0 + 512],
                                             t1[:], t2[:])

                for st in range(4):
                    pmm = ps_mm.tile([128, 512], F32, tag="mm")
                    for kt in range(NKT):
                        k0 = kt * 512
                        nc.tensor.matmul(
                            pmm[:],
                            xt[:, k0 + st * 128:k0 + (st + 1) * 128],
                            wv_all[:, k0:k0 + 512],
                            start=(kt == 0), stop=(kt == NKT - 1),
                        )
                    nc.vector.tensor_copy(v_t[sb * 4 + st][:], pmm[:])

            # wo loads: needed for phase C; emit now so DMA runs mid-kernel.
            wo_t = {}
            for h in range(HPC):
                for eb in range(4):
                    t = wopool.tile([128, 512], BF16, tag="wo")
                    nc.sync.dma_start(
                        t[:], wo[h * 128:(h + 1) * 128, eb * 512:(eb + 1) * 512]
                    )
                    wo_t[(h, eb)] = t

            # ---- phase B: attention per (i_block, head) ----
            for ib in range(NIB):
                i0 = ib * 512
                njt = 4 * ib + 4
                for h in range(HPC):
                    o_ps = ps_mm.tile([128, 512], F32, tag="mm")
                    l_ps = ps_l.tile([1, 512], F32, tag="l")

                    def emit_lpv(jt, voff, pt):
                        nc.tensor.matmul(
                            l_ps[:, voff:512], ones_col[:], pt[:, voff:512],
                            start=(jt == 0), stop=(jt == njt - 1),
                        )
                        nc.tensor.matmul(
                            o_ps[:, voff:512],
                            v_t[jt][:, h * 128:(h + 1) * 128],
                            pt[:, voff:512],
                            start=(jt == 0), stop=(jt == njt - 1),
                        )

                    # software-pipelined by one jt step: scores/exp for jt
                    # are emitted before L/PV of jt-1, so the tensor queue
                    # always has a scores matmul to run while exp(jt-1)
                    # finishes.
                    prev = None
                    for jt in range(njt):
                        j0 = jt * 128
                        voff = max(0, j0 - i0)
                        st_ps = ps_st.tile([128, 512], F32, tag="st")
                        diag = j0 >= i0
                        nc.tensor.matmul(
                            st_ps[:, voff:512],
                            kt_t[h][:, j0:j0 + 128],
                            qt_t[h][:, i0 + voff:i0 + 512],
                            start=True, stop=not diag,
                        )
                        if diag:
                            # causal mask added on the tensor engine:
                            # st[:, voff:voff+128] += triT^T @ I = tri
                            nc.tensor.matmul(
                                st_ps[:, voff:voff + 128],
                                triT_t[:], ident_t[:],
                                start=False, stop=True,
                            )
                        pt = ptpool.tile([128, 512], BF16, tag="pt")
                        nc.scalar.activation(
                            pt[:, voff:512], st_ps[:, voff:512],
                            mybir.ActivationFunctionType.Exp,
                        )
                        if prev is not None:
                            emit_lpv(*prev)
                        prev = (jt, voff, pt)
                    emit_lpv(*prev)

                    # normalization: copy L off PSUM (releases the L bank),
                    # broadcast on gpsimd, then full-width reciprocal on DVE
                    # ([1,512] reciprocal hits a 1-partition slow path).
                    lsb = lrpool.tile([1, 512], F32, tag="lrec")
                    nc.scalar.copy(lsb[:], l_ps[:])
                    bc = bcpool.tile([128, 512], F32, tag="bc")
                    nc.gpsimd.partition_broadcast(bc[:], lsb[:], channels=128)
                    rec = bcpool.tile([128, 512], F32, tag="rec")
                    nc.vector.reciprocal(rec[:], bc[:])
                    nc.vector.tensor_mul(ot_t[(h, ib)][:], o_ps[:], rec[:])

            # ---- phase C: output projection (partial over this head group) ----
            for stile in range(S // 128):
                ib, soff = stile // 4, (stile % 4) * 128
                co = copool.tile([128, 2048], BF16, tag="co")
                for eb in range(4):
                    pmm = ps_mm.tile([128, 512], F32, tag="mm")
                    for h in range(HPC):
                        nc.tensor.matmul(
                            pmm[:],
                            ot_t[(h, ib)][:, soff:soff + 128],
                            wo_t[(h, eb)][:],
                            start=(h == 0), stop=(h == HPC - 1),
                        )
                    if eb == 0:
                        nc.scalar.copy(co[:, eb * 512:(eb + 1) * 512], pmm[:])
                    else:
                        nc.vector.tensor_copy(
                            co[:, eb * 512:(eb + 1) * 512], pmm[:]
                        )
                nc.gpsimd.dma_start(
                    out[stile * 128:(stile + 1) * 128, :], co[:]
                )

    nc.compile()
    return nc


def _host_inputs(x, freqs_cos, freqs_sin, wq, wk, wv, wo):
    """Build the 8 per-core input maps (host-side sharding + layout prep)."""
    scale = 1.0 / np.sqrt(HD)
    # rope tables, d-major duplicated/interleaved: C[d,s]=cos[s,d//2];
    # S[2j,s]=-sin[s,j]; S[2j+1,s]=+sin[s,j]
    c = np.asarray(freqs_cos, dtype=np.float32)      # (S, HD/2)
    s = np.asarray(freqs_sin, dtype=np.float32)
    ropeC = np.repeat(c.T, 2, axis=0)                # (HD, S)
    ropeS = np.empty((HD, S), dtype=np.float32)
    ropeS[0::2] = -s.T
    ropeS[1::2] = s.T
    ropeC = ropeC.astype(BF)
    ropeS = ropeS.astype(BF)

    tri = np.where(
        np.arange(128)[:, None] <= np.arange(128)[None, :], 0.0, NEG
    ).astype(np.float32)
    triT = np.ascontiguousarray(tri.T).astype(BF)
    ident = np.eye(128, dtype=np.float32).astype(BF)
    pmat = np.zeros((128, 128), dtype=np.float32)
    idx = np.arange(128)
    pmat[idx, idx ^ 1] = 1.0
    pmat = pmat.astype(BF)

    xT = [np.ascontiguousarray(np.asarray(x[b]).T).astype(BF) for b in range(B)]
    wq = np.asarray(wq, dtype=np.float32)
    wk = np.asarray(wk, dtype=np.float32)
    wv = np.asarray(wv, dtype=np.float32)
    wo = np.asarray(wo, dtype=np.float32)

    in_maps = []
    for core in range(NCORES):
        b, g = core // GROUPS, core % GROUPS
        cols = slice(g * GD, (g + 1) * GD)
        in_maps.append({
            "xT": xT[b],
            "wq": np.ascontiguousarray(wq[:, cols] * scale).astype(BF),
            "wk": np.ascontiguousarray(wk[:, cols]).astype(BF),
            "wv": np.ascontiguousarray(wv[:, cols]).astype(BF),
            "wo": np.ascontiguousarray(wo[cols, :]).astype(BF),
            "ropeC": ropeC,
            "ropeS": ropeS,
            "triT": triT,
            "ident": ident,
            "pmat": pmat,
        })
    return in_maps


def _get_nc():
    if "nc" not in _CACHE:
        _CACHE["nc"] = _build()
    return _CACHE["nc"]


def run(inputs, trace=False, tmpdir=None):
    """Run on hardware; returns (full_output, BassKernelResults)."""
    nc = _get_nc()
    in_maps = _host_inputs(
        inputs["x"], inputs["freqs_cos"], inputs["freqs_sin"],
        inputs["wq"], inputs["wk"], inputs["wv"], inputs["wo"],
    )
    res = run_bass_kernel_spmd(
        nc, in_maps, core_ids=list(range(NCORES)), trace=trace, tmpdir=tmpdir
    )
    outs = [np.asarray(res.results[c]["out"], dtype=np.float32)
            for c in range(NCORES)]
    full = np.stack(
        [sum(outs[b * GROUPS + g] for g in range(GROUPS)) for b in range(B)],
        axis=0,
    )
    return full, res


def kernel(**inputs) -> np.ndarray:
    full, _ = run(inputs, trace=False)
    return full

